# revision 1
# baseline (speedup 1.0000x reference)
"""Cached multi-head attention (decode-append, S=4) on 8 Trainium2 NeuronCores.

Sharding: tensor-parallel over the 32 heads -> 4 heads per core.
  - Wq/Wk/Wv split on the output-feature (head) axis, Wo on the input axis.
  - Each core holds its heads' slice of the KV cache (positions 0..4095; the
    4 new positions are computed on-device from hidden_states).
  - Each core produces a partial [32, 4096] o_proj output; the "all-reduce"
    is a host-side sum of the 8 partials.

Per-core device kernel (fp16 streams, fp32 accumulation in PSUM):
  phase 1: x-stationary projections -> q/k/v token-major [32, 512], then PE
           transposes for feature-major qT/kT; per-batch v_new slices (with a
           ones column) via SBUF->SBUF DMA.
  phase 2: per (b, h): scores^T [128kv x 4tok] tiles via K^T-as-weights
           matmuls (32 kv tiles + 1 new-token tile with causal mask),
           exp via ACT (softmax max-subtraction skipped: |scores| <~ 6),
           PV with probsT-as-weights streaming V|ones [128, 129] -> the
           ones column accumulates the softmax denominator for free,
           normalize via reciprocal + per-token scalar mul, PE transpose
           to feature-major attnT.
  phase 3: o_proj with attnT-as-weights -> partial [32, 4096] fp32.
"""

import numpy as np

import concourse.bacc as bacc
import concourse.mybir as mybir
import concourse.tile as tile
from concourse.bass_utils import run_bass_kernel_spmd

N_CORES = 8
B, S, H = 8, 4, 4096
NH = 32                 # total heads
HPC = NH // N_CORES     # heads per core = 4
HD = H // NH            # head dim = 128
POS = 4096              # cache positions attended (rows >= POS are overwritten)
NT = POS // 128         # kv tiles per (b, h) = 32
NTOK = B * S            # 32 query tokens, token index = 4*b + s
KPC = HPC * HD          # per-core feature slice = 512
VW = HD + 1             # v tile width with ones column = 129
SCALE = HD ** -0.5
NEG_INF = -1e9

F16 = mybir.dt.float16
F32 = mybir.dt.float32


def build_nc():
    nc = bacc.Bacc("TRN2", target_bir_lowering=False)

    xT = nc.dram_tensor("xT", [128, NT * NTOK], F16, kind="ExternalInput")
    wq = nc.dram_tensor("wq", [128, NT * KPC], F16, kind="ExternalInput")
    wk = nc.dram_tensor("wk", [128, NT * KPC], F16, kind="ExternalInput")
    wv = nc.dram_tensor("wv", [128, NT * KPC], F16, kind="ExternalInput")
    wo = nc.dram_tensor("wo", [128, HPC * H], F16, kind="ExternalInput")
    kt = nc.dram_tensor("kt", [B, 128, HPC * POS], F16, kind="ExternalInput")
    v = nc.dram_tensor("v", [B, 128, HPC * NT * VW], F16, kind="ExternalInput")
    mask = nc.dram_tensor("mask", [S, S], F32, kind="ExternalInput")
    ident = nc.dram_tensor("ident", [32, 32], F16, kind="ExternalInput")
    out = nc.dram_tensor("out", [NTOK, H], F32, kind="ExternalOutput")

    with tile.TileContext(nc) as tc:
        _body(tc, xT.ap(), wq.ap(), wk.ap(), wv.ap(), wo.ap(), kt.ap(), v.ap(),
              mask.ap(), ident.ap(), out.ap())
    nc.compile()
    return nc


def _body(tc, xT, wq, wk, wv, wo, kt, v, mask, ident, out):
    nc = tc.nc
    from contextlib import ExitStack
    Exp = mybir.ActivationFunctionType.Exp
    HT = NT // 2
    TAG_BUFS = {"scores": 3, "out4": 2, "sn": 2, "tpose": 1}
    ctx = ExitStack()
    with ctx:
        consts = ctx.enter_context(tc.tile_pool(name="consts", bufs=1))
        persist = ctx.enter_context(tc.tile_pool(name="persist", bufs=1))
        wpool = ctx.enter_context(tc.tile_pool(name="wpool", bufs=2))
        kvpool = ctx.enter_context(tc.tile_pool(name="kvpool", bufs=3))
        smpool = ctx.enter_context(tc.tile_pool(name="smpool", bufs=2))
        ps = ctx.enter_context(tc.tile_pool(name="ps", bufs=2, space="PSUM"))

        # ---- DMA preamble: interleave first kv chunks with weight halves ----
        xT_sb = persist.tile([128, NT * NTOK], F16)
        nc.sync.dma_start(out=xT_sb, in_=xT)
        mask_sb = consts.tile([S, S], F32)
        nc.sync.dma_start(out=mask_sb, in_=mask)
        id_sb = consts.tile([32, 32], F16)
        nc.sync.dma_start(out=id_sb, in_=ident)

        def w_halves(w_dram, name):
            tiles = []
            for half in range(2):
                wh = wpool.tile([128, HT * KPC], F16, tag="w", name=f"{name}{half}")
                nc.sync.dma_start(
                    out=wh, in_=w_dram[:, HT * KPC * half: HT * KPC * (half + 1)])
                tiles.append(wh)
            return tiles

        kvch = {}

        def fetch_kv(b, hp):
            ktch = kvpool.tile([128, 2 * POS], F16, tag="kt", name=f"kt{b}{hp}")
            nc.sync.dma_start(out=ktch, in_=kt[b][:, 2 * POS * hp: 2 * POS * (hp + 1)])
            vch = kvpool.tile([128, 2 * NT * VW], F16, tag="v", name=f"v{b}{hp}")
            nc.sync.dma_start(out=vch, in_=v[b][:, 2 * NT * VW * hp: 2 * NT * VW * (hp + 1)])
            kvch[(b, hp)] = (ktch, vch)

        wqh = w_halves(wq, "wq")
        fetch_kv(0, 0)
        wkh = w_halves(wk, "wk")
        fetch_kv(1, 0)
        wvh = w_halves(wv, "wv")
        fetch_kv(2, 0)
        # o_proj weights on the SWDGE ring, overlapping the attention stream
        wo_a = wpool.tile([128, 2 * H], F16, tag="w")
        nc.gpsimd.dma_start(out=wo_a, in_=wo[:, 0: 2 * H])
        wo_b = wpool.tile([128, 2 * H], F16, tag="w")
        nc.gpsimd.dma_start(out=wo_b, in_=wo[:, 2 * H: 4 * H])

        # ---- phase 1: projections (x-stationary, token-major) ----
        qT_sb = persist.tile([128, HPC * NTOK], F16)
        kT_sb = persist.tile([128, HPC * NTOK], F16)
        attnT_sb = persist.tile([128, HPC * NTOK], F16)
        vnew_sb = [persist.tile([S, HPC * VW], F16, name=f"vnew{b}") for b in range(B)]

        q_tok = persist.tile([NTOK, KPC], F16)
        k_tok = persist.tile([NTOK, KPC], F16)
        v_tok = persist.tile([NTOK, KPC], F16)
        for whs, tok_dst, tagp in ((wqh, q_tok, "scores"), (wkh, k_tok, "out4"),
                                   (wvh, v_tok, "tpose")):
            pp = ps.tile([NTOK, KPC], F32, tag=tagp, name=f"pp_{tagp}", bufs=TAG_BUFS[tagp])
            for half in range(2):
                for tt in range(HT):
                    t = HT * half + tt
                    nc.tensor.matmul(
                        pp, lhsT=xT_sb[:, NTOK * t: NTOK * (t + 1)],
                        rhs=whs[half][:, KPC * tt: KPC * (tt + 1)],
                        start=(t == 0), stop=(t == NT - 1))
            nc.scalar.copy(out=tok_dst, in_=pp)

        # feature-major qT/kT via PE transpose of [32, 128] chunks
        for src_t, dst in ((q_tok, qT_sb), (k_tok, kT_sb)):
            for m in range(HPC):
                tp = ps.tile([128, NTOK], F16, tag="tpose", bufs=1)
                nc.tensor.transpose(tp, in_=src_t[:, HD * m: HD * (m + 1)], identity=id_sb)
                nc.scalar.copy(out=dst[:, NTOK * m: NTOK * (m + 1)], in_=tp)

        # per-batch v_new [4, 4*129] (ones col per head) at partitions 0..3
        for b in range(B):
            vb = vnew_sb[b].rearrange("p (h d) -> p h d", d=VW)
            nc.vector.memset(vb[:, :, HD:VW], 1.0)
            nc.gpsimd.dma_start(
                out=vb[:, :, 0:HD],
                in_=v_tok[S * b: S * (b + 1), :].rearrange("p (h d) -> p h d", d=HD),
            )

        # ---- phase 2: attention (head-pair major: o_proj can start halfway) ----
        o_part = persist.tile([NTOK, H], F32)
        o_all = persist.tile([NTOK, H], F32)
        for hp in range(HPC // 2):
            for b in range(B):
                last = (hp == HPC // 2 - 1 and b == B - 1)
                if last:
                    # final chunk at per-head granularity: the very last head's
                    # work starts as soon as its own 1MB slices land
                    kts, vs = [], []
                    for hh in range(2):
                        k1 = kvpool.tile([128, POS], F16, tag="kt", name=f"ktL{hh}")
                        nc.sync.dma_start(
                            out=k1, in_=kt[b][:, POS * (2 * hp + hh): POS * (2 * hp + hh + 1)])
                        v1 = kvpool.tile([128, NT * VW], F16, tag="v", name=f"vL{hh}")
                        nc.sync.dma_start(
                            out=v1, in_=v[b][:, NT * VW * (2 * hp + hh): NT * VW * (2 * hp + hh + 1)])
                        kts.append(k1)
                        vs.append(v1)
                else:
                    if (b, hp) not in kvch:
                        fetch_kv(b, hp)
                    ktch, vch = kvch[(b, hp)]
                for hh in range(2):
                    h = 2 * hp + hh
                    if last:
                        ktch, vch = kts[hh], vs[hh]
                    koff = 0 if last else POS * hh
                    voff = 0 if last else NT * VW * hh
                    col = NTOK * h + S * b  # (head, batch) column in qT/kT/attnT
                    scores = ps.tile([128, NT * S], F32, tag="scores", bufs=3)
                    for t in range(NT):
                        nc.tensor.matmul(
                            scores[:, S * t: S * (t + 1)],
                            lhsT=ktch[:, koff + 128 * t: koff + 128 * t + 128],
                            rhs=qT_sb[:, col: col + S],
                            start=True, stop=True,
                        )
                    probs = smpool.tile([128, NT * S], F16, tag="probs")
                    nc.scalar.activation(out=probs, in_=scores, func=Exp, scale=SCALE)
                    # new-token scores [4 kv_new, 4 tok] + causal mask (separate
                    # tiles so the cache pipeline doesn't wait on k/v proj)
                    sn = ps.tile([S, S], F32, tag="sn", bufs=2)
                    nc.tensor.matmul(sn, lhsT=kT_sb[:, col: col + S],
                                     rhs=qT_sb[:, col: col + S], start=True, stop=True)
                    nc.vector.tensor_add(out=sn, in0=sn, in1=mask_sb)
                    pn = smpool.tile([S, S], F16, tag="pn")
                    nc.scalar.activation(out=pn, in_=sn, func=Exp, scale=SCALE)
                    # PV: probsT stationary, V|ones streaming; col 128 = denom
                    out4 = ps.tile([S, VW], F32, tag="out4", bufs=2)
                    for t in range(NT):
                        nc.tensor.matmul(
                            out4,
                            lhsT=probs[:, S * t: S * (t + 1)],
                            rhs=vch[:, voff + VW * t: voff + VW * (t + 1)],
                            start=(t == 0), stop=False,
                        )
                    nc.tensor.matmul(
                        out4, lhsT=pn,
                        rhs=vnew_sb[b][:, VW * h: VW * (h + 1)],
                        start=False, stop=True,
                    )
                    rec = smpool.tile([S, 1], F32, tag="rec")
                    nc.vector.reciprocal(out=rec, in_=out4[0:S, HD:VW])
                    atok = smpool.tile([S, HD], F16, tag="atok")
                    nc.vector.tensor_scalar_mul(atok, in0=out4[0:S, 0:HD], scalar1=rec)
                    tp = ps.tile([128, S], F16, tag="tpose", bufs=1)
                    nc.tensor.transpose(tp, in_=atok, identity=id_sb[0:S, 0:S])
                    nc.scalar.copy(out=attnT_sb[:, col: col + S], in_=tp)

            # o_proj for this head pair: hp==0 stages into o_part, hp==1 adds
            for n in range(H // 512):
                op = ps.tile([NTOK, 512], F32, tag="scores", bufs=3)
                for jj in range(2):
                    j = 2 * hp + jj
                    wo_half = wo_a if hp == 0 else wo_b
                    nc.tensor.matmul(
                        op,
                        lhsT=attnT_sb[:, NTOK * j: NTOK * (j + 1)],
                        rhs=wo_half[:, H * jj + 512 * n: H * jj + 512 * (n + 1)],
                        start=(jj == 0), stop=(jj == 1),
                    )
                if hp == 0:
                    nc.scalar.copy(out=o_part[:, 512 * n: 512 * (n + 1)], in_=op)
                else:
                    nc.vector.tensor_add(out=o_all[:, 512 * n: 512 * (n + 1)], in0=op,
                                         in1=o_part[:, 512 * n: 512 * (n + 1)])
        nc.sync.dma_start(out=out, in_=o_all)


# ---------------------------------------------------------------------------
# host side
# ---------------------------------------------------------------------------

def build_core_inputs(hidden_states, Wq, Wk, Wv, Wo, key_cache, value_cache):
    """Shard + lay out the full inputs into the 8 per-core DRAM images."""
    tokens = np.ascontiguousarray(hidden_states.reshape(NTOK, H))
    xT = tokens.T.astype(np.float16)                       # [4096, 32]
    xT_sb = np.ascontiguousarray(
        xT.reshape(NT, 128, NTOK).transpose(1, 0, 2)).reshape(128, NT * NTOK)

    WqT = Wq.T.astype(np.float16)                          # [in=4096, out=4096]
    WkT = Wk.T.astype(np.float16)
    WvT = Wv.T.astype(np.float16)
    WoT = Wo.T.astype(np.float16)                          # [in, out]
    K16 = key_cache[:, :, :POS, :].astype(np.float16)      # [B, NH, POS, HD]
    V16 = value_cache[:, :, :POS, :].astype(np.float16)

    mask = np.where(np.arange(S)[:, None] > np.arange(S)[None, :],
                    np.float32(NEG_INF), np.float32(0.0))
    ident = np.eye(32, dtype=np.float16)

    in_maps = []
    for c in range(N_CORES):
        cs = slice(KPC * c, KPC * (c + 1))
        hs = slice(HPC * c, HPC * (c + 1))

        def wlayout(WT):
            a = np.ascontiguousarray(WT[:, cs])            # [4096, 512]
            return np.ascontiguousarray(
                a.reshape(NT, 128, KPC).transpose(1, 0, 2)).reshape(128, NT * KPC)

        wo_c = np.ascontiguousarray(WoT[cs, :])            # [512, 4096]
        wo_c = np.ascontiguousarray(
            wo_c.reshape(HPC, 128, H).transpose(1, 0, 2)).reshape(128, HPC * H)

        kt_c = np.ascontiguousarray(
            K16[:, hs].transpose(0, 3, 1, 2)).reshape(B, 128, HPC * POS)
        v_p = V16[:, hs].reshape(B, HPC, NT, 128, HD)      # [b, h, t, kv, d]
        v_aug = np.ones((B, HPC, NT, 128, VW), np.float16)
        v_aug[..., :HD] = v_p
        v_c = np.ascontiguousarray(
            v_aug.transpose(0, 3, 1, 2, 4)).reshape(B, 128, HPC * NT * VW)

        in_maps.append({
            "xT": xT_sb, "wq": wlayout(WqT), "wk": wlayout(WkT),
            "wv": wlayout(WvT), "wo": wo_c, "kt": kt_c, "v": v_c,
            "mask": mask, "ident": ident,
        })
    return in_maps


def numpy_core_kernel(m):
    """Numpy mirror of the device dataflow for one core (layout validation)."""
    f = np.float32
    f16 = np.float16
    xT_sb = m["xT"].astype(f)
    xT = xT_sb.reshape(128, NT, NTOK).transpose(1, 0, 2).reshape(H, NTOK)

    def unw(w):
        return w.astype(f).reshape(128, NT, KPC).transpose(1, 0, 2).reshape(H, KPC)

    qT = (unw(m["wq"]).T @ xT).astype(f16).astype(f)      # [512 feat, 32 tok]
    kT = (unw(m["wk"]).T @ xT).astype(f16).astype(f)
    vnew = (unw(m["wv"]).T @ xT).T.astype(f16).astype(f)  # [32 tok, 512 feat]

    attnT = np.zeros((KPC, NTOK), f)
    for b in range(B):
        for h in range(HPC):
            colsl = slice(S * b, S * b + S)
            KTbh = m["kt"][b].astype(f)[:, POS * h: POS * (h + 1)]   # [hd, kv]
            scoresT = KTbh.T @ qT[HD * h: HD * (h + 1), colsl]       # [kv, 4]
            snew = kT[HD * h: HD * (h + 1), colsl].T @ qT[HD * h: HD * (h + 1), colsl]
            snew = snew + m["mask"]                                  # [j, s]
            pr = np.exp(SCALE * scoresT).astype(f16).astype(f)
            prnew = np.exp(SCALE * snew).astype(f16).astype(f)
            den = pr.sum(axis=0) + prnew.sum(axis=0)
            vb = m["v"][b].astype(f)[:, NT * VW * h: NT * VW * (h + 1)]
            V_bh = vb.reshape(128, NT, VW)[:, :, :HD].transpose(1, 0, 2).reshape(POS, HD)
            ou = V_bh.T @ pr + vnew[S * b: S * b + S, HD * h: HD * (h + 1)].T @ prnew
            attnT[HD * h: HD * (h + 1), colsl] = (ou / den).astype(f16)
    woc = m["wo"].astype(f).reshape(128, HPC, H).transpose(1, 0, 2).reshape(KPC, H)
    return (attnT.astype(f16).astype(f).T @ woc).astype(np.float32)


_NC_CACHE = None


def get_nc():
    global _NC_CACHE
    if _NC_CACHE is None:
        _NC_CACHE = build_nc()
    return _NC_CACHE


def run_on_hw(inputs, trace=False, trace_cores=None):
    position = int(inputs["position"])
    assert position == POS, position
    in_maps = build_core_inputs(
        np.asarray(inputs["hidden_states"]), np.asarray(inputs["Wq"]),
        np.asarray(inputs["Wk"]), np.asarray(inputs["Wv"]), np.asarray(inputs["Wo"]),
        np.asarray(inputs["key_cache"]), np.asarray(inputs["value_cache"]))
    nc = get_nc()
    res = run_bass_kernel_spmd(nc, in_maps, core_ids=list(range(N_CORES)),
                               trace=trace, trace_cores=trace_cores)
    partial = np.zeros((NTOK, H), np.float64)
    for c in range(N_CORES):
        partial += res.results[c]["out"].astype(np.float64)
    out = partial.astype(np.float32).reshape(B, S, H)
    return out, res


def kernel(**inputs) -> np.ndarray:
    out, _ = run_on_hw(inputs, trace=False)
    return out



# revision 11
# speedup vs baseline: 1.3383x; 1.3383x over previous
"""Cached multi-head attention (decode-append, S=4) on 8 Trainium2 NeuronCores.

Sharding: tensor-parallel over the 32 heads -> 4 heads per core.
  - Wq/Wk/Wv split on the output-feature (head) axis, Wo on the input axis.
  - Each core holds its heads' slice of the KV cache.
  - Each core produces a partial output; the all-reduce is a host-side sum.

Precision/layout strategy (v2):
  - K and V caches stored as fp8 e3m4 (4-bit mantissa), pre-scaled on the
    host into e3m4's range (alpha_k, alpha_v).  The PE eats them directly as
    stationary weights (mixed fp8 x fp16 matmuls) -- no on-chip dequant.
    Descales fold into the exp scale constant (1/alpha_k^2, since Wq is also
    host-scaled by alpha_k) and host-folded Wo rows (1/alpha_v).  Wk/Wv are
    stored as e3m4 * alpha_{k,v} so the on-chip k_new/v_new match the cache
    scaling exactly; their quantization noise only touches the 4 new
    positions (~0.1% of the attention mass).
  - Scores: lhsT = K^T-tile [hd=128, kv=128] e3m4 stationary (~48ns/LD),
    rhs = qT [hd, 4 tok] fp16 -> scores^T [kv, tok] in PSUM, s-major cols.
  - PV flipped: lhsT = V-tile [kv=128, feat=128] e3m4, rhs = probs [kv, 4]
    -> out4 [feat, tok] accumulated over kv tiles in PSUM.  No transposes,
    no 129-column moving streams.
  - Denominators: per (b,h) one matmul ones[128,1]^T @ probs -> [1,128],
    DVE strided reduce -> [1,4]; new-token part via ones32 over pn_all.
    Normalize at the end: reciprocal [1,128], PE row-broadcast, one DVE
    tensor_mul over all 128 (head, token) columns.
  - q/k projections flipped: W^T tiles [in=128, out=128] stationary, xT
    moving -> qT/kT feature-major directly.  v-proj x-stationary -> v_tok
    token-major for the new-token PV; pn_all [32,32] with a block-diagonal
    causal mask kills cross-batch terms in one matmul per head.
  - o_proj flipped: Wo^T tiles stationary, attnT moving -> out^T [H, tok];
    host transposes and sums partials across cores.
"""

import numpy as np
import ml_dtypes

import concourse.bacc as bacc
import concourse.mybir as mybir
import concourse.tile as tile
from concourse.bass_utils import run_bass_kernel_spmd

N_CORES = 8
B, S, H = 8, 4, 4096
NH = 32                 # total heads
HPC = NH // N_CORES     # heads per core = 4
HD = H // NH            # head dim = 128
POS = 4096              # cache positions attended
NT = POS // 128         # kv tiles per (b, h) = 32
NTOK = B * S            # 32 query tokens, token index = 4*b + s
KPC = HPC * HD          # per-core feature slice = 512
SCALE = HD ** -0.5
NEG = -1e9

F16 = mybir.dt.float16
F32 = mybir.dt.float32
E3 = mybir.dt.float8e3
E3NP = ml_dtypes.float8_e3m4

E3_TARGET = 14.0        # e3m4 max normal is 15.5; leave rounding headroom
V_FP16 = True           # V cache in fp16 (safe accuracy) vs e3m4 (fast DMA)


def build_nc(alpha_k, gk, gv):
    """alpha_k bakes into the exp scale, gk/gv into the k/v copy scales."""
    nc = bacc.Bacc("TRN2", target_bir_lowering=False)
    VDT = F16 if V_FP16 else E3

    xT = nc.dram_tensor("xT", [128, NT * NTOK], F16, kind="ExternalInput")
    wq = nc.dram_tensor("wq", [128, NT * KPC], F16, kind="ExternalInput")
    wk = nc.dram_tensor("wk", [128, NT * KPC], E3, kind="ExternalInput")
    wv = nc.dram_tensor("wv", [128, NT * KPC], E3, kind="ExternalInput")
    wo = nc.dram_tensor("wo", [128, HPC * H], F16, kind="ExternalInput")
    kt = nc.dram_tensor("kt", [B, 128, HPC * POS], E3, kind="ExternalInput")
    v = nc.dram_tensor("v", [B, 128, HPC * POS], VDT, kind="ExternalInput")
    mask = nc.dram_tensor("mask", [NTOK, NTOK], F32, kind="ExternalInput")
    out = nc.dram_tensor("out", [128, NT * NTOK], F32, kind="ExternalOutput")

    with tile.TileContext(nc) as tc:
        _body(tc, xT.ap(), wq.ap(), wk.ap(), wv.ap(), wo.ap(), kt.ap(), v.ap(),
              mask.ap(), out.ap(), alpha_k, gk, gv)
    nc.compile()
    return nc


def _body(tc, xT, wq, wk, wv, wo, kt, v, mask, out, alpha_k, gk, gv):
    nc = tc.nc
    from contextlib import ExitStack
    Exp = mybir.ActivationFunctionType.Exp
    Copy = mybir.ActivationFunctionType.Copy
    VDT = F16 if V_FP16 else E3
    ESCALE = SCALE / (alpha_k * alpha_k)  # scores carry alpha_k^2

    ctx = ExitStack()
    with ctx:
        consts = ctx.enter_context(tc.tile_pool(name="consts", bufs=1))
        persist = ctx.enter_context(tc.tile_pool(name="persist", bufs=1))
        wpool = ctx.enter_context(tc.tile_pool(name="wpool", bufs=1))
        kvpool = ctx.enter_context(tc.tile_pool(name="kvpool", bufs=3))
        smpool = ctx.enter_context(tc.tile_pool(name="smpool", bufs=3))
        ps = ctx.enter_context(tc.tile_pool(name="ps", bufs=2, space="PSUM"))

        # ---- constants / persistent state ----
        ones128 = consts.tile([128, 1], F16)
        nc.vector.memset(ones128, 1.0)
        ones32 = consts.tile([32, 1], F16)
        nc.vector.memset(ones32, 1.0)
        ones_row = consts.tile([1, HPC * NTOK], F32)
        nc.vector.memset(ones_row, 1.0)
        mask_sb = consts.tile([NTOK, NTOK], F32)
        nc.sync.dma_start(out=mask_sb, in_=mask)

        xT_sb = persist.tile([128, NT * NTOK], F16)
        nc.sync.dma_start(out=xT_sb, in_=xT)
        wq_sb = wpool.tile([128, NT * KPC], F16)
        nc.sync.dma_start(out=wq_sb, in_=wq)
        wk_sb = wpool.tile([128, NT * KPC], E3)
        nc.gpsimd.dma_start(out=wk_sb, in_=wk)
        wv_sb = wpool.tile([128, NT * KPC], E3)
        nc.gpsimd.dma_start(out=wv_sb, in_=wv)
        wo_sb = wpool.tile([128, HPC * H], F16)
        nc.gpsimd.dma_start(out=wo_sb, in_=wo)

        # KV prefetch ring
        kvch = {}

        def fetch_kv(i):
            b, h = divmod(i, HPC)
            kc = kvpool.tile([128, POS], E3, tag="kt", name=f"kt{i}")
            nc.sync.dma_start(out=kc, in_=kt[b][:, POS * h: POS * (h + 1)])
            vc = kvpool.tile([128, POS], VDT, tag="v", name=f"v{i}")
            nc.sync.dma_start(out=vc, in_=v[b][:, POS * h: POS * (h + 1)])
            kvch[i] = (kc, vc)

        fetch_kv(0)
        fetch_kv(1)

        # ---- phase 1: projections ----
        # q/k flipped: W^T tile [in 128, feat 128] stationary, xT moving
        qT_sb = persist.tile([128, HPC * NTOK], F16)
        kT_sb = persist.tile([128, HPC * NTOK], F16)
        v_tok = persist.tile([NTOK, KPC], F16)

        for w_sb, dst, dsc in ((wq_sb, qT_sb, None), (wk_sb, kT_sb, 1.0 / gk)):
            for h in range(HPC):
                pp = ps.tile([128, NTOK], F32, tag="sc", name=f"pj{h}", bufs=3)
                for t in range(NT):
                    nc.tensor.matmul(
                        pp,
                        lhsT=w_sb[:, KPC * t + HD * h: KPC * t + HD * (h + 1)],
                        rhs=xT_sb[:, NTOK * t: NTOK * (t + 1)],
                        start=(t == 0), stop=(t == NT - 1))
                if dsc is None:
                    nc.scalar.copy(out=dst[:, NTOK * h: NTOK * (h + 1)], in_=pp)
                else:
                    nc.scalar.activation(out=dst[:, NTOK * h: NTOK * (h + 1)],
                                         in_=pp, func=Copy, scale=float(dsc))

        # v-proj x-stationary: xT tile stationary, Wv moving -> [tok, feat]
        vp = ps.tile([NTOK, KPC], F32, tag="sc", bufs=3)
        for t in range(NT):
            nc.tensor.matmul(
                vp, lhsT=xT_sb[:, NTOK * t: NTOK * (t + 1)],
                rhs=wv_sb[:, KPC * t: KPC * (t + 1)],
                start=(t == 0), stop=(t == NT - 1))
        nc.scalar.activation(out=v_tok, in_=vp, func=Copy, scale=float(1.0 / gv))

        # pn_all per head: [32 kv-new, 32 tok] with block-diag causal mask
        pn_all = [persist.tile([NTOK, NTOK], F16, name=f"pn{h}") for h in range(HPC)]
        dn_sb = persist.tile([1, HPC * NTOK], F32)
        for h in range(HPC):
            sn = ps.tile([NTOK, NTOK], F32, tag="dd", bufs=3)
            nc.tensor.matmul(sn, lhsT=kT_sb[:, NTOK * h: NTOK * (h + 1)],
                             rhs=qT_sb[:, NTOK * h: NTOK * (h + 1)],
                             start=True, stop=True)
            snm = smpool.tile([NTOK, NTOK], F32, tag="snm", bufs=2)
            nc.vector.tensor_add(out=snm, in0=sn, in1=mask_sb)
            nc.scalar.activation(out=pn_all[h], in_=snm, func=Exp, scale=ESCALE)
            dnp = ps.tile([1, NTOK], F32, tag="dd", bufs=3)
            nc.tensor.matmul(dnp, lhsT=ones32, rhs=pn_all[h], start=True, stop=True)
            nc.scalar.copy(out=dn_sb[:, NTOK * h: NTOK * (h + 1)], in_=dnp)

        # ---- phase 2: attention over the cache, per (b, h) ----
        out4_all = persist.tile([128, HPC * NTOK], F32)
        den_all = persist.tile([1, HPC * NTOK], F32)

        for i in range(B * HPC):
            b, h = divmod(i, HPC)
            if i + 2 < B * HPC:
                fetch_kv(i + 2)
            kc, vc = kvch.pop(i)
            col = NTOK * h + S * b  # (head, token) column in qT/out4/den

            # scores^T [kv 128, (s,t) 128] s-major: col = s*32 + t
            sc_ps = ps.tile([128, S * NT], F32, tag="sc", bufs=3)
            sc3 = sc_ps.rearrange("p (s t) -> p s t", t=NT)
            for t in range(NT):
                nc.tensor.matmul(
                    sc3[:, :, t: t + 1],
                    lhsT=kc[:, 128 * t: 128 * (t + 1)],
                    rhs=qT_sb[:, col: col + S],
                    start=True, stop=True)
            probs = smpool.tile([128, S * NT], F16, tag="probs", bufs=3)
            nc.scalar.activation(out=probs, in_=sc_ps, func=Exp, scale=ESCALE)
            pr3 = probs.rearrange("p (s t) -> p s t", t=NT)

            # denominator: ones^T @ probs -> [1, 128], reduce t, add new part
            dd = ps.tile([1, S * NT], F32, tag="dd", bufs=3)
            nc.tensor.matmul(dd, lhsT=ones128, rhs=probs, start=True, stop=True)
            dtmp = smpool.tile([1, S], F32, tag="dtmp", bufs=2)
            nc.vector.reduce_sum(
                out=dtmp, in_=dd.rearrange("p (s t) -> p s t", t=NT),
                axis=mybir.AxisListType.X)
            nc.vector.tensor_add(out=den_all[:, col: col + S], in0=dtmp,
                                 in1=dn_sb[:, col: col + S])

            # PV flipped: V-tile stationary [kv, feat], probs moving [kv, 4]
            o4 = ps.tile([128, S], F32, tag="o4", bufs=2)
            for t in range(NT):
                nc.tensor.matmul(
                    o4, lhsT=vc[:, 128 * t: 128 * (t + 1)],
                    rhs=pr3[:, :, t: t + 1],
                    start=(t == 0), stop=False)
            nc.tensor.matmul(
                o4, lhsT=v_tok[:, HD * h: HD * (h + 1)],
                rhs=pn_all[h][:, S * b: S * (b + 1)],
                start=False, stop=True)
            nc.scalar.copy(out=out4_all[:, col: col + S], in_=o4)

        # ---- phase 3: normalize + o_proj ----
        rec = smpool.tile([1, HPC * NTOK], F32, tag="rec", bufs=1)
        nc.vector.reciprocal(out=rec, in_=den_all)
        rb_ps = ps.tile([128, HPC * NTOK], F32, tag="o4", bufs=2)
        nc.tensor.matmul(rb_ps, lhsT=ones_row, rhs=rec, start=True, stop=True)
        attnT = persist.tile([128, HPC * NTOK], F16)
        nc.vector.tensor_mul(out=attnT, in0=out4_all, in1=rb_ps)

        # o_proj flipped: Wo^T tile [in-feat 128, out 128] stationary,
        # attnT [in-feat, 32 tok] moving -> out^T chunk [128, 32]
        o_all = persist.tile([128, NT * NTOK], F32)
        for oc in range(NT):
            op = ps.tile([128, NTOK], F32, tag="sc", bufs=3)
            for h in range(HPC):
                nc.tensor.matmul(
                    op, lhsT=wo_sb[:, H * h + 128 * oc: H * h + 128 * (oc + 1)],
                    rhs=attnT[:, NTOK * h: NTOK * (h + 1)],
                    start=(h == 0), stop=(h == HPC - 1))
            nc.scalar.copy(out=o_all[:, NTOK * oc: NTOK * (oc + 1)], in_=op)
        nc.sync.dma_start(out=out, in_=o_all)


# ---------------------------------------------------------------------------
# host side
# ---------------------------------------------------------------------------

def _scales(key_cache, value_cache):
    ak = E3_TARGET / max(float(np.abs(key_cache[:, :, :POS]).max()), 1e-6)
    if V_FP16:
        av = 1.0
    else:
        av = E3_TARGET / max(float(np.abs(value_cache[:, :, :POS]).max()), 1e-6)
    return ak, av


def build_core_inputs(hidden_states, Wq, Wk, Wv, Wo, key_cache, value_cache):
    """Shard + lay out the full inputs into the 8 per-core DRAM images."""
    ak, av = _scales(key_cache, value_cache)

    tokens = np.ascontiguousarray(hidden_states.reshape(NTOK, H))
    xT = tokens.T.astype(np.float16)                       # [4096, 32]
    xT_sb = np.ascontiguousarray(
        xT.reshape(NT, 128, NTOK).transpose(1, 0, 2)).reshape(128, NT * NTOK)

    WqT = Wq.T.astype(np.float32) * ak                     # [in, out] * ak
    # k_new/v_new must match the cache scaling (ak / av); gk, gv lift the
    # e3m4-stored weights out of the denormal range and are divided back out
    # by the on-chip psum->sbuf copy scales.
    WkTs = Wk.T.astype(np.float32) * ak
    gk = E3_TARGET / max(float(np.abs(WkTs).max()), 1e-30)
    WkT = WkTs * gk
    WvTs = Wv.T.astype(np.float32) * av
    gv = E3_TARGET / max(float(np.abs(WvTs).max()), 1e-30)
    WvT = WvTs * gv
    WoT = Wo.T.astype(np.float32) / av                     # undo av after PV

    def wlayout_flip(WT, dt):
        # [4096 in, 512 out] -> [128, (t_in 32, 512)] tiles [in128, out...]
        a = np.ascontiguousarray(WT).reshape(NT, 128, KPC)
        return np.ascontiguousarray(a.transpose(1, 0, 2)).reshape(128, NT * KPC).astype(dt)

    K8 = (key_cache[:, :, :POS].astype(np.float32) * ak).astype(E3NP)
    VNP = np.float16 if V_FP16 else E3NP
    V8 = (value_cache[:, :, :POS].astype(np.float32) * av).astype(VNP)

    # block-diagonal causal mask for pn_all [kv-new i, tok j]
    mask = np.full((NTOK, NTOK), np.float32(NEG), np.float32)
    for b in range(B):
        for i_ in range(S):
            for j in range(i_, S):
                mask[S * b + i_, S * b + j] = 0.0

    in_maps = []
    for c in range(N_CORES):
        cs = slice(KPC * c, KPC * (c + 1))
        hs = slice(HPC * c, HPC * (c + 1))

        wq_c = wlayout_flip(WqT[:, cs], np.float16)
        wk_c = wlayout_flip(WkT[:, cs], E3NP)
        wv_c = wlayout_flip(WvT[:, cs], E3NP)

        # wo: [512 in, 4096 out] rows slice -> [128, (h 4, oc*128+o)]
        wo_c = np.ascontiguousarray(WoT[cs, :]).reshape(HPC, 128, H)
        wo_c = np.ascontiguousarray(wo_c.transpose(1, 0, 2)).reshape(128, HPC * H)
        wo_c = wo_c.astype(np.float16)

        # kt: K^T per (b,h): [hd 128, kv 4096]
        kt_c = np.ascontiguousarray(
            K8[:, hs].transpose(0, 3, 1, 2)).reshape(B, 128, HPC * POS)
        # v: [kv-in-tile 128, (h, t, f)]
        v_c = np.ascontiguousarray(
            V8[:, hs].reshape(B, HPC, NT, 128, HD).transpose(0, 3, 1, 2, 4)
        ).reshape(B, 128, HPC * POS)

        in_maps.append({
            "xT": xT_sb, "wq": wq_c, "wk": wk_c, "wv": wv_c, "wo": wo_c,
            "kt": kt_c, "v": v_c, "mask": mask,
        })
    return in_maps, (ak, gk, gv)


def numpy_core_kernel(m, scales):
    """Numpy mirror of the device dataflow for one core (layout validation)."""
    ak, gk, gv = scales
    f = np.float32
    f16 = np.float16
    escale = SCALE / (ak * ak)
    xT = m["xT"].astype(f).reshape(128, NT, NTOK).transpose(1, 0, 2).reshape(H, NTOK)

    def unw(w):
        return w.astype(f).reshape(128, NT, KPC).transpose(1, 0, 2).reshape(H, KPC)

    qT = (unw(m["wq"]).T @ xT).astype(f16).astype(f)      # [512 feat, 32 tok]
    kT = (unw(m["wk"]).T @ xT / gk).astype(f16).astype(f)
    v_tok = (xT.T @ unw(m["wv"]) / gv).astype(f16).astype(f)   # [32 tok, 512]

    pn_all = []
    for h in range(HPC):
        sn = kT[HD * h: HD * (h + 1), :].T @ qT[HD * h: HD * (h + 1), :]
        pn = np.exp(escale * (sn + m["mask"])).astype(f16).astype(f)
        pn_all.append(pn)

    out4 = np.zeros((128, HPC * NTOK), f)
    den = np.zeros((1, HPC * NTOK), f)
    for b in range(B):
        for h in range(HPC):
            col = NTOK * h + S * b
            KTbh = m["kt"][b].astype(f)[:, POS * h: POS * (h + 1)]   # [hd, kv]
            scT = KTbh.T @ qT[HD * h: HD * (h + 1), S * b: S * b + S]  # [kv, 4]
            pr = np.exp(escale * scT).astype(f16).astype(f)
            Vbh = m["v"][b].astype(f)[:, POS * h: POS * (h + 1)]
            Vb = Vbh.reshape(128, NT, HD)
            o4 = np.zeros((HD, S), f)
            for t in range(NT):
                o4 += Vb[:, t, :].T @ pr[128 * t: 128 * (t + 1), :]
            o4 += v_tok[:, HD * h: HD * (h + 1)].T @ pn_all[h][:, S * b: S * (b + 1)]
            d = pr.sum(axis=0) + pn_all[h][:, S * b: S * (b + 1)].sum(axis=0)
            out4[:, col: col + S] = o4
            den[0, col: col + S] = d

    rec = 1.0 / den
    attnT = (out4 * rec).astype(f16).astype(f)            # [128, (h tok)]

    woc = m["wo"].astype(f).reshape(128, HPC, H).transpose(1, 0, 2).reshape(KPC, H)
    outT = np.zeros((H, NTOK), f)
    for oc in range(NT):
        acc = np.zeros((128, NTOK), f)
        for h in range(HPC):
            acc += woc[128 * h: 128 * (h + 1), 128 * oc: 128 * (oc + 1)].T \
                @ attnT[:, NTOK * h: NTOK * (h + 1)]
        outT[128 * oc: 128 * (oc + 1)] = acc
    # device o_all layout: [128, (oc, tok)]
    return np.ascontiguousarray(
        outT.reshape(NT, 128, NTOK).transpose(1, 0, 2)).reshape(128, NT * NTOK)


def host_unpack(o_all):
    """[128, (oc 32, tok 32)] -> [NTOK, H]"""
    a = o_all.reshape(128, NT, NTOK).transpose(1, 0, 2).reshape(H, NTOK)
    return a.T


_NC_CACHE = {}


def get_nc(scales):
    ak, gk, gv = scales
    key = (round(float(ak), 6), round(float(gk), 6), round(float(gv), 6))
    if key not in _NC_CACHE:
        _NC_CACHE[key] = build_nc(ak, gk, gv)
    return _NC_CACHE[key]


def run_on_hw(inputs, trace=False, trace_cores=None):
    position = int(inputs["position"])
    assert position == POS, position
    in_maps, scales = build_core_inputs(
        np.asarray(inputs["hidden_states"]), np.asarray(inputs["Wq"]),
        np.asarray(inputs["Wk"]), np.asarray(inputs["Wv"]), np.asarray(inputs["Wo"]),
        np.asarray(inputs["key_cache"]), np.asarray(inputs["value_cache"]))
    nc = get_nc(scales)
    res = run_bass_kernel_spmd(nc, in_maps, core_ids=list(range(N_CORES)),
                               trace=trace, trace_cores=trace_cores)
    partial = np.zeros((NTOK, H), np.float64)
    for c in range(N_CORES):
        partial += host_unpack(res.results[c]["out"].astype(np.float64))
    out = partial.astype(np.float32).reshape(B, S, H)
    return out, res


def kernel(**inputs) -> np.ndarray:
    out, _ = run_on_hw(inputs, trace=False)
    return out
